# revision 1
# baseline (speedup 1.0000x reference)
"""Trainium2 Bass kernel for nn_AttentionBlock (GroupNorm + 1x1-conv attention).

Contract: kernel(**inputs) takes FULL unsharded inputs (numpy, shapes as in
setup_inputs) and returns the FULL output. Internally shards batch (32) over
8 NeuronCores (4 batch elements per core), params replicated.

Math per batch element (faithful to the reference's raw channels-last
reshape): with q,k,v the (hw=1024, c=512) projection outputs, the raw
reshape to (c, hw) produces matrices whose row r is the concat of pixel
rows 2r and 2r+1.  We compute
    S^T = K2^T Q2    (contraction over the 512 "pixel-pair" axis)
    P^T = exp(S^T / sqrt(c))          (no max-subtraction; scores are O(1))
    Z   = colsum(P) via ones-matmul, 1/Z applied in the O-drain
    O^T = (P^T as lhsT) @ V2^T        -> (hw', c') layout
then un-reshape via an even/odd interleave copy and apply the final conv +
residual.
"""

import sys

sys.path.insert(0, "/opt/trn_rl_repo")

from contextlib import ExitStack

import numpy as np

import concourse.bass as bass
import concourse.tile as tile
from concourse import bacc, mybir
from concourse.bass_utils import run_bass_kernel_spmd

B, H, W, C = 32, 32, 32, 512
HW = H * W  # 1024
NCORES = 8
NB = B // NCORES  # 4 batch elements per core
P = 128
GROUPS = 32
EPS = 1e-6
F32 = mybir.dt.float32
BF16 = mybir.dt.bfloat16

CT = C // P  # 4 channel tiles
MT = HW // P  # 8 pixel tiles


def build_bass(nb: int = NB):
    # Bacc (not raw Bass): its finalize() runs generate_event_semaphores,
    # which splits multi-wait instructions to satisfy the 1-wait HW limit.
    nc = bacc.Bacc()

    # x and the four weight matrices arrive pre-cast to bf16 from the host
    # (everything on-device consumes bf16; skips the on-device cast chain).
    x_in = nc.declare_dram_parameter("xbf16", [nb, HW, C], BF16, isOutput=False)
    gamma_in = nc.declare_dram_parameter("gn_gamma", [C], F32, isOutput=False)
    beta_in = nc.declare_dram_parameter("gn_beta", [C], F32, isOutput=False)
    wq_in = nc.declare_dram_parameter("wq", [C, C], BF16, isOutput=False)
    bq_in = nc.declare_dram_parameter("bq", [C], F32, isOutput=False)
    wk_in = nc.declare_dram_parameter("wk", [C, C], BF16, isOutput=False)
    bk_in = nc.declare_dram_parameter("bk", [C], F32, isOutput=False)
    wv_in = nc.declare_dram_parameter("wv", [C, C], BF16, isOutput=False)
    bv_in = nc.declare_dram_parameter("bv", [C], F32, isOutput=False)
    wo_in = nc.declare_dram_parameter("wo", [C, C], BF16, isOutput=False)
    bo_in = nc.declare_dram_parameter("bo", [C], F32, isOutput=False)
    # Output in bf16 (upcast to f32 on the host): halves output DMA traffic.
    out_ext = nc.declare_dram_parameter("out", [nb, HW, C], BF16, isOutput=True)

    # Block-diagonal group-averaging matrix: gmat[i, j] = 1/16 iff same group.
    gs = C // GROUPS  # 16 channels per group
    gnp = np.zeros((P, P), dtype=np.float32)
    for g in range(P // gs):
        gnp[g * gs : (g + 1) * gs, g * gs : (g + 1) * gs] = 1.0 / gs
    gmat_dram = nc.inline_tensor(gnp, name="gmat")

    with tile.TileContext(nc) as tc, ExitStack() as ctx:
        ep = ctx.enter_context

        consts = ep(tc.tile_pool(name="consts", bufs=1))
        wtmp = ep(tc.tile_pool(name="wtmp", bufs=1))
        p_xb = ep(tc.tile_pool(name="p_xb", bufs=2))       # bf16 x; also residual
        p_xT = ep(tc.tile_pool(name="p_xT", bufs=CT))
        p_xn = ep(tc.tile_pool(name="p_xn", bufs=8))
        p_st = ep(tc.tile_pool(name="p_st", bufs=4))
        p_q2 = ep(tc.tile_pool(name="p_q2", bufs=2))
        p_pt = ep(tc.tile_pool(name="p_pt", bufs=MT + CT))
        p_v = ep(tc.tile_pool(name="p_v", bufs=2 * CT + 2))
        p_op = ep(tc.tile_pool(name="p_op", bufs=2 * CT + 2))
        p_z = ep(tc.tile_pool(name="p_z", bufs=4))
        p_out = ep(tc.tile_pool(name="p_out", bufs=4))

        # PSUM: 8 banks total.  pp(2) + ps(2) + po1(2) + po2(1) + psm(1) = 8
        pp = ep(tc.tile_pool(name="pp", bufs=2, space="PSUM"))
        ps = ep(tc.tile_pool(name="ps", bufs=2, space="PSUM"))
        po1 = ep(tc.tile_pool(name="po1", bufs=2, space="PSUM"))
        po2 = ep(tc.tile_pool(name="po2", bufs=1, space="PSUM"))
        psm = ep(tc.tile_pool(name="psm", bufs=1, space="PSUM"))

        # ---- small constants first so GroupNorm of elem 0 can start early ----
        gcol = consts.tile([P, CT], F32, name="gamma")
        nc.sync.dma_start(gcol, gamma_in.rearrange("(t p) -> p t", p=P))
        bcol = consts.tile([P, CT], F32, name="beta")
        nc.sync.dma_start(bcol, beta_in.rearrange("(t p) -> p t", p=P))
        bv_col = consts.tile([P, CT], F32, name="bv")
        nc.sync.dma_start(bv_col, bv_in.rearrange("(t p) -> p t", p=P))
        gmat_sb = consts.tile([P, P], F32, name="gmat")
        nc.sync.dma_start(gmat_sb, gmat_dram[:, :])
        eps_sb = consts.tile([P, 1], F32, name="eps")
        nc.vector.memset(eps_sb, EPS)
        zero_sb = consts.tile([P, 1], F32, name="zero")
        nc.vector.memset(zero_sb, 0.0)

        inv_sqrt_c = float(C) ** -0.5
        w_sb = {}

        for ib in range(nb):
            # pixel-major views of this element's x slab, (128, 8, 512)-tiled
            xb_v = x_in[ib].rearrange("(t p) c -> p t c", p=P)

            # ---- transpose-load x^T straight from the DRAM input; the
            # residual copy of x is loaded after the compute-gating DMAs ----
            xT = []
            for ct in range(CT):
                tt = p_xT.tile([P, HW], BF16, name="xT")
                nc.sync.dma_start_transpose(tt, x_in[ib][:, ct * P : (ct + 1) * P])
                xT.append(tt)

            if ib == 0:
                # weights (already bf16) + broadcast row biases; loaded after
                # elem 0's transposes so GroupNorm isn't stuck behind them
                for name, wext in (
                    ("q", wq_in), ("k", wk_in), ("v", wv_in), ("o", wo_in)
                ):
                    wb = consts.tile([P, CT, C], BF16, name=f"w_{name}")
                    nc.sync.dma_start(wb, wext.rearrange("(kt p) c -> p kt c", p=P))
                    w_sb[name] = wb
                bq_f32 = wtmp.tile([P, C], F32, name="bqf", tag="wf")
                nc.sync.dma_start(bq_f32, bq_in[None, :].to_broadcast((P, C)))
                bq_sb = consts.tile([P, C], BF16, name="bq")
                nc.vector.tensor_copy(bq_sb, bq_f32)
                bk_f32 = wtmp.tile([P, C], F32, name="bkf", tag="wf")
                nc.sync.dma_start(bk_f32, bk_in[None, :].to_broadcast((P, C)))
                bk_sb = consts.tile([P, C], BF16, name="bk")
                nc.vector.tensor_copy(bk_sb, bk_f32)
                bo_sb = consts.tile([P, C], F32, name="bo")
                nc.sync.dma_start(bo_sb, bo_in[None, :].to_broadcast((P, C)))

            # bf16 x kept in SBUF for the final residual add (not needed
            # until the last phase, so loaded behind weights/transposes)
            xallb = p_xb.tile([P, MT, C], BF16, name="xallb")
            nc.sync.dma_start(xallb, xb_v)

            # ---- GroupNorm ----
            xnT = []
            for ct in range(CT):
                stats = p_st.tile([P, 2, 6], F32, name="bnstats")
                nc.vector.bn_stats(stats[:, 0, :], xT[ct][:, 0:512])
                nc.vector.bn_stats(stats[:, 1, :], xT[ct][:, 512:1024])
                mv = p_st.tile([P, 2], F32, name="mv")
                nc.vector.bn_aggr(mv, stats)
                # msq = [mean_ch, var_ch + mean_ch^2] = [mean_ch, E[x^2]_ch]
                msq = p_st.tile([P, 2], F32, name="msq")
                nc.vector.tensor_copy(msq[:, 0:1], mv[:, 0:1])
                nc.vector.tensor_mul(msq[:, 1:2], mv[:, 0:1], mv[:, 0:1])
                nc.vector.tensor_add(msq[:, 1:2], msq[:, 1:2], mv[:, 1:2])
                # group-average across the 16 channels of each group
                gps = psm.tile([P, 2], F32, name="gps")
                nc.tensor.matmul(gps, lhsT=gmat_sb, rhs=msq, start=True, stop=True)
                mu = p_st.tile([P, 1], F32, name="mu")
                nc.vector.tensor_copy(mu, gps[:, 0:1])
                varg = p_st.tile([P, 1], F32, name="varg")
                nc.vector.tensor_mul(varg, mu, mu)
                nc.vector.tensor_tensor(
                    varg, gps[:, 1:2], varg, mybir.AluOpType.subtract
                )
                sd = p_st.tile([P, 1], F32, name="sd")
                nc.scalar.activation(
                    sd, varg, mybir.ActivationFunctionType.Sqrt, bias=eps_sb[:, 0:1]
                )
                nc.vector.reciprocal(sd, sd)
                scale_col = p_st.tile([P, 1], F32, name="scale_col")
                nc.vector.tensor_mul(scale_col, sd, gcol[:, ct : ct + 1])
                shift_col = p_st.tile([P, 1], F32, name="shift_col")
                nc.vector.tensor_mul(shift_col, mu, scale_col)
                nc.vector.tensor_tensor(
                    shift_col, bcol[:, ct : ct + 1], shift_col, mybir.AluOpType.subtract
                )
                xn = p_xn.tile([P, HW], BF16, name="xnT")
                nc.gpsimd.tensor_scalar(
                    out=xn,
                    in0=xT[ct],
                    scalar1=scale_col,
                    scalar2=shift_col,
                    op0=mybir.AluOpType.mult,
                    op1=mybir.AluOpType.add,
                )
                xnT.append(xn)

            # ---- q, k projections, written DIRECTLY in the raw-reshape
            # (Q2/K2) layout: output tile (rt, u) covers pixels
            # {2r+u : r in [128rt, 128rt+128)} -- a stride-2 column slice of
            # xnT as lhsT makes the matmul's output partition = Q2 row.
            # Q2[r, u*512+ch] = q[2r+u, ch] lands at q2sb[:, rt, u*512:+512].
            q2sb = p_q2.tile([P, CT, HW], BF16, name="q2", tag="q2")
            k2sb = p_q2.tile([P, CT, HW], BF16, name="k2", tag="k2")
            xnv = [
                xnT[kt].rearrange("p (rt m x) -> p rt x m", rt=CT, x=2)
                for kt in range(CT)
            ]
            for rt in range(CT):
                for u in range(2):
                    for big, wname, brow in ((q2sb, "q", bq_sb), (k2sb, "k", bk_sb)):
                        acc = pp.tile([P, C], F32, name="proj_ps")
                        for kt in range(CT):
                            nc.tensor.matmul(
                                acc,
                                lhsT=xnv[kt][:, rt, u, :],
                                rhs=w_sb[wname][:, kt, :],
                                start=(kt == 0),
                                stop=(kt == CT - 1),
                            )
                        nc.vector.tensor_add(
                            big[:, rt, u * 512 : (u + 1) * 512], acc, brow
                        )

            # ---- v projection (channel-major) with even/odd pixel split ----
            veven = []
            vodd = []
            for ct in range(CT):
                ve = p_v.tile([P, 513], BF16, name="veven")
                vo = p_v.tile([P, 513], BF16, name="vodd")
                nc.vector.memset(ve[:, 512:513], 1.0)
                nc.vector.memset(vo[:, 512:513], 1.0)
                for n in range(2):
                    acc = pp.tile([P, 512], F32, name="proj_ps")
                    for kt in range(CT):
                        nc.tensor.matmul(
                            acc,
                            lhsT=w_sb["v"][:, kt, ct * P : (ct + 1) * P],
                            rhs=xnT[kt][:, n * 512 : (n + 1) * 512],
                            start=(kt == 0),
                            stop=(kt == CT - 1),
                        )
                    pv = acc.rearrange("p (m two) -> p two m", two=2)
                    nc.vector.tensor_scalar_add(
                        ve[:, n * 256 : (n + 1) * 256], pv[:, 0, :],
                        bv_col[:, ct : ct + 1],
                    )
                    nc.vector.tensor_scalar_add(
                        vo[:, n * 256 : (n + 1) * 256], pv[:, 1, :],
                        bv_col[:, ct : ct + 1],
                    )
                veven.append(ve)
                vodd.append(vo)

            # ---- S^T = K2^T Q2, then P^T = exp(S^T/sqrt(c)) ----
            PT = [p_pt.tile([P, HW], BF16, name="pt") for _ in range(MT)]
            for bt in range(MT):
                for at in range(2):
                    sps = ps.tile([P, 512], F32, name="s_ps")
                    for rt in range(CT):
                        nc.tensor.matmul(
                            sps,
                            lhsT=k2sb[:, rt, bt * P : (bt + 1) * P],
                            rhs=q2sb[:, rt, at * 512 : (at + 1) * 512],
                            start=(rt == 0),
                            stop=(rt == CT - 1),
                        )
                    nc.scalar.activation(
                        PT[bt][:, at * 512 : (at + 1) * 512],
                        sps,
                        mybir.ActivationFunctionType.Exp,
                        bias=zero_sb[:, 0:1],
                        scale=inv_sqrt_c,
                    )

            # ---- O^T = P @ [V2^T | 1]: the appended ones column makes the
            # second accumulator's last column the softmax denominator Z for
            # exactly this output tile's rows, already in per-partition form.
            # The drain divides by it while undoing the raw reshape. ----
            opT = [p_op.tile([P, HW], BF16, name="opT") for _ in range(CT)]
            for am in range(MT):
                ops1 = po1.tile([P, 256], F32, name="o_ps1")
                ops2 = po2.tile([P, 257], F32, name="o_ps2")
                for bt in range(MT):
                    rhs = veven[bt] if bt < CT else vodd[bt - CT]
                    lhsT = PT[bt][:, am * P : (am + 1) * P]
                    nc.tensor.matmul(
                        ops1, lhsT=lhsT, rhs=rhs[:, 0:256],
                        start=(bt == 0), stop=(bt == MT - 1),
                    )
                    nc.tensor.matmul(
                        ops2, lhsT=lhsT, rhs=rhs[:, 256:513],
                        start=(bt == 0), stop=(bt == MT - 1),
                    )
                zinv = p_z.tile([P, 1], F32, name="zinv")
                nc.vector.reciprocal(zinv, ops2[:, 256:257])
                cht, u = am % CT, am // CT
                dst = opT[cht].rearrange("p (m two) -> p two m", two=2)[:, u, :]
                nc.vector.tensor_scalar_mul(dst[:, 0:256], ops1, zinv)
                nc.vector.tensor_scalar_mul(dst[:, 256:512], ops2[:, 0:256], zinv)

            # ---- final projection + bias + residual (bf16 copy of x) ----
            for mt in range(MT):
                acc = pp.tile([P, C], F32, name="proj_ps")
                for kt in range(CT):
                    nc.tensor.matmul(
                        acc,
                        lhsT=opT[kt][:, mt * P : (mt + 1) * P],
                        rhs=w_sb["o"][:, kt, :],
                        start=(kt == 0),
                        stop=(kt == CT - 1),
                    )
                osb = p_out.tile([P, C], BF16, name="osb")
                nc.vector.tensor_add(osb, acc, bo_sb)
                nc.vector.tensor_add(osb, osb, xallb[:, mt, :])
                nc.sync.dma_start(out_ext[ib, mt * P : (mt + 1) * P, :], osb)

    nc.finalize()
    return nc


_nc_cache = {}


def get_nc(nb: int = NB):
    if nb not in _nc_cache:
        _nc_cache[nb] = build_bass(nb)
    return _nc_cache[nb]


def kernel(x, gn_gamma, gn_beta, wq, bq, wk, bk, wv, bv, wo, bo, **run_kwargs):
    import ml_dtypes

    bf16 = ml_dtypes.bfloat16
    xb = np.ascontiguousarray(
        np.asarray(x, dtype=np.float32).astype(bf16)
    ).reshape(B, HW, C)
    params = {
        "gn_gamma": np.ascontiguousarray(np.asarray(gn_gamma, dtype=np.float32)),
        "gn_beta": np.ascontiguousarray(np.asarray(gn_beta, dtype=np.float32)),
        "wq": np.ascontiguousarray(np.asarray(wq, dtype=np.float32).astype(bf16)),
        "bq": np.ascontiguousarray(np.asarray(bq, dtype=np.float32)),
        "wk": np.ascontiguousarray(np.asarray(wk, dtype=np.float32).astype(bf16)),
        "bk": np.ascontiguousarray(np.asarray(bk, dtype=np.float32)),
        "wv": np.ascontiguousarray(np.asarray(wv, dtype=np.float32).astype(bf16)),
        "bv": np.ascontiguousarray(np.asarray(bv, dtype=np.float32)),
        "wo": np.ascontiguousarray(np.asarray(wo, dtype=np.float32).astype(bf16)),
        "bo": np.ascontiguousarray(np.asarray(bo, dtype=np.float32)),
    }
    nc = get_nc(NB)
    in_maps = [
        {"xbf16": xb[i * NB : (i + 1) * NB], **params} for i in range(NCORES)
    ]
    res = run_bass_kernel_spmd(nc, in_maps, core_ids=list(range(NCORES)), **run_kwargs)
    global last_results
    last_results = res
    out = np.concatenate([res.results[i]["out"] for i in range(NCORES)], axis=0)
    return out.reshape(B, H, W, C).astype(np.float32)


last_results = None


if __name__ == "__main__":
    nc = build_bass(NB)
    print("build + compile OK")

